# revision 8
# baseline (speedup 1.0000x reference)
"""Trainium2 Bass kernel: CMAFM fusion (segment min/max stats -> attention
MLPs -> gated 2-layer MLP over voxels), data-parallel over the batch axis.

Sharding: batch b -> NeuronCore b (batch_idx is sorted, B == n_cores == 8).
Each core computes its own batch's feature min/max stats locally, runs the
tiny attention MLPs on-device, folds the per-feature gating into the first
fused-MLP weight matrix, and runs the big MLP over its voxels. No
collectives: every voxel's gating row is core-local by construction.

v2 layout: host pre-converts voxel data to bf16; the device ingests it with
XBAR DMA-transpose straight into feature-major resident SBUF (no PE
transposes, no PSUM staging copies). Segment min/max run as bf16
TensorTensor chains (2x DVE mode). The fused MLP runs in bf16 with 2-bank
PSUM tiles so each PSUM->SBUF relu-copy is a single wide op, alternating
between DVE and Act. Output is stored bf16 and upconverted on host (relu
commutes with bf16 rounding).
"""

import os
import sys

import numpy as np

for _p in ("/opt/trn_rl_repo",):
    if os.path.isdir(_p) and _p not in sys.path:
        sys.path.append(_p)

B = 8
L = 128
C = 128
OUT = 256
CA = 512
H = 170
VT = 512    # voxels per compute tile
CW = 2048   # voxels per load/stat chunk

STATS_GP = 0  # how many of the 4 stat streams run on GpSimd (Pool)

_cache = {}


def _build(S_pad, mm_dt="bf16", stats_gp=STATS_GP, reps=1):
    from contextlib import ExitStack

    import concourse.bacc as bacc
    import concourse.mybir as mybir
    import concourse.tile as tile

    f32 = mybir.dt.float32
    bf16 = mybir.dt.bfloat16
    Alu = mybir.AluOpType
    Act = mybir.ActivationFunctionType

    n_tiles = S_pad // VT
    cw = min(CW, S_pad)
    # chunk spans (start, width), widths multiples of VT, first is `cw` wide
    spans = []
    v0 = 0
    while v0 < S_pad:
        w = min(cw, S_pad - v0)
        spans.append((v0, w))
        v0 += w

    nc = bacc.Bacc("TRN2", target_bir_lowering=False, debug=False, num_devices=B)
    lidar = nc.dram_tensor("lidar", [S_pad, L], bf16, kind="ExternalInput").ap()
    cam = nc.dram_tensor("cam", [S_pad, C], bf16, kind="ExternalInput").ap()
    wl1 = nc.dram_tensor("W_l1", [CA, H], f32, kind="ExternalInput").ap()
    wl2 = nc.dram_tensor("W_l2", [H, L], f32, kind="ExternalInput").ap()
    wc1 = nc.dram_tensor("W_c1", [CA, H], f32, kind="ExternalInput").ap()
    wc2 = nc.dram_tensor("W_c2", [H, C], f32, kind="ExternalInput").ap()
    wf1 = nc.dram_tensor("W_f1", [2 * L, OUT], f32, kind="ExternalInput").ap()
    wf2 = nc.dram_tensor("W_f2", [OUT, OUT], f32, kind="ExternalInput").ap()
    out = nc.dram_tensor("out", [S_pad, OUT], bf16, kind="ExternalOutput").ap()

    with tile.TileContext(nc) as tc, ExitStack() as ctx:
        wpool = ctx.enter_context(tc.tile_pool(name="weights", bufs=1))
        respool = ctx.enter_context(tc.tile_pool(name="res", bufs=1))
        statpool = ctx.enter_context(tc.tile_pool(name="stat", bufs=1))

        # --- weights to SBUF ---
        wf1_s = wpool.tile([128, 2, OUT], f32)
        nc.sync.dma_start(wf1_s[:], wf1.rearrange("(a p) o -> p a o", p=128))
        wf2_stage = wpool.tile([128, 2, OUT], f32)
        nc.sync.dma_start(wf2_stage[:], wf2.rearrange("(a p) o -> p a o", p=128))
        wf2_s = wpool.tile([128, 2, OUT], bf16)
        nc.vector.tensor_copy(wf2_s[:], wf2_stage[:])
        w1e_s = wpool.tile([128, 2, OUT], bf16)
        wl1_s = wpool.tile([128, 4, H], f32)
        nc.sync.dma_start(wl1_s[:], wl1.rearrange("(a p) h -> p a h", p=128))
        wc1_s = wpool.tile([128, 4, H], f32)
        nc.sync.dma_start(wc1_s[:], wc1.rearrange("(a p) h -> p a h", p=128))
        wl2a_s = wpool.tile([128, L], f32)
        nc.sync.dma_start(wl2a_s[:], wl2[0:128, :])
        wl2b_s = wpool.tile([H - 128, L], f32)
        nc.sync.dma_start(wl2b_s[:], wl2[128:H, :])
        wc2a_s = wpool.tile([128, C], f32)
        nc.sync.dma_start(wc2a_s[:], wc2[0:128, :])
        wc2b_s = wpool.tile([H - 128, C], f32)
        nc.sync.dma_start(wc2b_s[:], wc2[128:H, :])

        # resident feature-major voxel data
        xres = {
            "l": respool.tile([128, S_pad], bf16, name="xres_l", tag="xres_l"),
            "c": respool.tile([128, S_pad], bf16, name="xres_c", tag="xres_c"),
        }
        accbuf = {}
        for key in ("min_l", "max_l", "min_c", "max_c"):
            accbuf[key] = statpool.tile([128, cw], bf16, name="a" + key, tag="a" + key)

        # stream engine assignment: optionally push trailing streams to GpSimd
        streams = ["min_l", "max_l", "min_c", "max_c"]
        eng_of = {}
        for i, key in enumerate(streams):
            eng_of[key] = nc.gpsimd if i >= len(streams) - stats_gp else nc.vector

        for _rep in range(reps):
            rctx = ctx.enter_context(ExitStack())
            # ---- pass 1: transpose-loads + min/max stat chains ----
            for si, (v0, w) in enumerate(spans):
                for which, src in (("l", lidar), ("c", cam)):
                    dst = xres[which][:, v0 : v0 + w]
                    nc.sync.dma_start(dst, src[v0 : v0 + w, :], transpose=True)
                    for statname, op in (("min", Alu.min), ("max", Alu.max)):
                        key = statname + "_" + which
                        acc = accbuf[key][:, 0:w]
                        eng = eng_of[key]
                        if si == 0:
                            eng.tensor_copy(acc, dst)
                        else:
                            eng.tensor_tensor(out=acc, in0=acc, in1=dst, op=op)

            # ---- finalize stats: fold acc to VT, then reduce to [128,1] ----
            stat = {}
            for key in ("min_l", "max_l", "min_c", "max_c"):
                op = Alu.min if key.startswith("min") else Alu.max
                eng = eng_of[key]
                w = cw
                while w > VT:
                    h = w // 2
                    eng.tensor_tensor(
                        out=accbuf[key][:, 0:h],
                        in0=accbuf[key][:, 0:h],
                        in1=accbuf[key][:, h:w],
                        op=op,
                    )
                    w = h
                s = statpool.tile([128, 1], f32, tag="stat" + key)
                nc.vector.tensor_reduce(
                    s[:], accbuf[key][:, 0:w], axis=mybir.AxisListType.X, op=op
                )
                stat[key] = s
            cat_chunks = [stat["min_l"], stat["max_l"], stat["min_c"], stat["max_c"]]

            # ---- tiny attention MLPs + gating fold ----
            with tc.tile_pool(name="pstiny", bufs=1, space="PSUM") as pstiny:

                def tiny_mlp(w1_s, w2a_s, w2b_s, name):
                    h1_sb = []
                    for tag, mo, mn in (("h1a", 0, 128), ("h1b", 128, H - 128)):
                        ps = pstiny.tile([mn, 1], f32, tag=tag + name)
                        for k in range(4):
                            nc.tensor.matmul(
                                ps[:],
                                w1_s[:, k, mo : mo + mn],
                                cat_chunks[k][:],
                                start=(k == 0),
                                stop=(k == 3),
                            )
                        hs = statpool.tile([mn, 1], f32, tag=tag + "s" + name)
                        nc.scalar.activation(hs[:], ps[:], Act.Relu)
                        h1_sb.append(hs)
                    att_ps = pstiny.tile([128, 1], f32, tag="attps" + name)
                    nc.tensor.matmul(
                        att_ps[:], w2a_s[:], h1_sb[0][:], start=True, stop=False
                    )
                    nc.tensor.matmul(
                        att_ps[:], w2b_s[:], h1_sb[1][:], start=False, stop=True
                    )
                    att_r = statpool.tile([128, 1], f32, tag="attr" + name)
                    nc.scalar.activation(att_r[:], att_ps[:], Act.Relu)
                    att = statpool.tile([128, 1], f32, tag="att" + name)
                    nc.scalar.activation(att[:], att_r[:], Act.Sigmoid)
                    return att

                att_l = tiny_mlp(wl1_s, wl2a_s, wl2b_s, "l")
                att_c = tiny_mlp(wc1_s, wc2a_s, wc2b_s, "c")

            nc.vector.tensor_scalar(
                out=w1e_s[:, 0, :], in0=wf1_s[:, 0, :], scalar1=att_l[:],
                scalar2=None, op0=Alu.mult,
            )
            nc.vector.tensor_scalar(
                out=w1e_s[:, 1, :], in0=wf1_s[:, 1, :], scalar1=att_c[:],
                scalar2=None, op0=Alu.mult,
            )

            # ---- pass 2: big gated MLP (software-pipelined PE stream) ----
            psl1 = rctx.enter_context(tc.tile_pool(name="psl1", bufs=2, space="PSUM"))
            psl2 = rctx.enter_context(tc.tile_pool(name="psl2", bufs=2, space="PSUM"))
            h1pool = rctx.enter_context(tc.tile_pool(name="h1", bufs=2))
            outpool = rctx.enter_context(tc.tile_pool(name="outp", bufs=2))

            def emit_l1(t):
                """L1 matmuls for tile t -> 2-bank PSUM tile + dual relu-copies."""
                xt_l = xres["l"][:, t * VT : (t + 1) * VT]
                xt_c = xres["c"][:, t * VT : (t + 1) * VT]
                ps1 = psl1.tile([128, 2 * VT], f32, tag="psl1")
                h1a = h1pool.tile([128, VT], bf16, tag="h1a")
                h1b = h1pool.tile([128, VT], bf16, tag="h1b")
                for m in range(2):
                    sl = ps1[:, m * VT : (m + 1) * VT]
                    nc.tensor.matmul(
                        sl, w1e_s[:, 0, m * 128 : (m + 1) * 128], xt_l,
                        start=True, stop=False,
                    )
                    nc.tensor.matmul(
                        sl, w1e_s[:, 1, m * 128 : (m + 1) * 128], xt_c,
                        start=False, stop=True,
                    )
                    # copy overlaps the other half's matmuls; DVE owns h1
                    if m == 0:
                        nc.vector.tensor_scalar_max(h1a[:], ps1[:, 0:VT], 0.0)
                    else:
                        nc.vector.tensor_scalar_max(h1b[:], ps1[:, VT : 2 * VT], 0.0)
                return h1a, h1b

            def emit_l2(t, h1a, h1b, ob2):
                ps2 = psl2.tile([128, 4, OUT], f32, tag="psl2")
                for g in range(2):
                    for h in range(2):
                        v = g * 2 + h
                        sl = ps2[:, g * 2 + h, :]
                        nc.tensor.matmul(
                            sl, h1a[:, v * 128 : (v + 1) * 128], wf2_s[:, 0, :],
                            start=True, stop=False,
                        )
                        nc.tensor.matmul(
                            sl, h1b[:, v * 128 : (v + 1) * 128], wf2_s[:, 1, :],
                            start=False, stop=True,
                        )
                    # Act owns ob copies (keeps each engine's queue inversion-free)
                    obs = ob2[:, t % 2, g * 2 : (g + 1) * 2, :]
                    nc.scalar.activation(obs, ps2[:, g * 2 : (g + 1) * 2, :], Act.Relu)

            ob2 = None
            h1cur = emit_l1(0)
            for t in range(n_tiles):
                h1next = emit_l1(t + 1) if t + 1 < n_tiles else None
                if t % 2 == 0:
                    ob2 = outpool.tile([128, 2, 4, OUT], bf16, tag="ob2")
                emit_l2(t, *h1cur, ob2)
                h1cur = h1next
                if t % 2 == 1:
                    r0 = (t - 1) * VT
                    nc.sync.dma_start(
                        out[r0 : r0 + 2 * VT, :].rearrange("(a p) f -> p a f", p=128),
                        ob2[:].rearrange("p b a f -> p (b a) f"),
                    )
            if n_tiles % 2 == 1:
                r0 = (n_tiles - 1) * VT
                nc.sync.dma_start(
                    out[r0 : r0 + VT, :].rearrange("(a p) f -> p a f", p=128),
                    ob2[:, 0, :, :],
                )

            rctx.close()

    nc.compile()
    return nc


def _get_program(S_pad):
    key = (S_pad, STATS_GP)
    if key not in _cache:
        _cache[key] = _build(S_pad, stats_gp=STATS_GP)
    return _cache[key]


def _to_bf16(a):
    import ml_dtypes

    return np.asarray(a, np.float32).astype(ml_dtypes.bfloat16)


def _from_bf16(a):
    return np.asarray(a).astype(np.float32)


def shard_inputs(lidar, cam, batch_idx, W_l1, W_l2, W_c1, W_c2, W_f1, W_f2):
    """Split by batch (batch_idx sorted), pad with replicated real rows."""
    lidar = _to_bf16(lidar)
    cam = _to_bf16(cam)
    batch_idx = np.asarray(batch_idx)
    bounds = np.searchsorted(batch_idx, np.arange(B + 1))
    sizes = np.diff(bounds)
    S_pad = int(-(-max(int(sizes.max()), 1) // VT) * VT)
    weights = {
        "W_l1": np.ascontiguousarray(W_l1, np.float32),
        "W_l2": np.ascontiguousarray(W_l2, np.float32),
        "W_c1": np.ascontiguousarray(W_c1, np.float32),
        "W_c2": np.ascontiguousarray(W_c2, np.float32),
        "W_f1": np.ascontiguousarray(W_f1, np.float32),
        "W_f2": np.ascontiguousarray(W_f2, np.float32),
    }
    in_maps = []
    for b in range(B):
        s0, s1 = int(bounds[b]), int(bounds[b + 1])
        n = s1 - s0
        l = np.empty((S_pad, L), lidar.dtype)
        c = np.empty((S_pad, C), cam.dtype)
        if n > 0:
            l[:n] = lidar[s0:s1]
            c[:n] = cam[s0:s1]
            l[n:] = lidar[s1 - 1]
            c[n:] = cam[s1 - 1]
        else:
            l[:] = 0
            c[:] = 0
        in_maps.append({"lidar": l, "cam": c, **weights})
    return in_maps, bounds, sizes, S_pad


def kernel(lidar, cam, batch_idx, W_l1, W_l2, W_c1, W_c2, W_f1, W_f2):
    from concourse.bass_utils import run_bass_kernel_spmd

    in_maps, bounds, sizes, S_pad = shard_inputs(
        lidar, cam, batch_idx, W_l1, W_l2, W_c1, W_c2, W_f1, W_f2
    )
    nc = _get_program(S_pad)
    res = run_bass_kernel_spmd(nc, in_maps, core_ids=list(range(B)))
    N = lidar.shape[0]
    out_full = np.empty((N, OUT), np.float32)
    for b in range(B):
        s0, s1 = int(bounds[b]), int(bounds[b + 1])
        if s1 > s0:
            out_full[s0:s1] = _from_bf16(res.results[b]["out"][: s1 - s0])
    return out_full


# revision 9
# speedup vs baseline: 2.9213x; 2.9213x over previous
"""Trainium2 Bass kernel: CMAFM fusion (segment min/max stats -> attention
MLPs -> gated 2-layer MLP over voxels), data-parallel over the batch axis.

Sharding: batch b -> NeuronCore b (batch_idx is sorted, B == n_cores == 8).
Each core computes its own batch's feature min/max stats locally, runs the
tiny attention MLPs on-device, folds the per-feature gating into the first
fused-MLP weight matrix, and runs the big MLP over its voxels. No
collectives: every voxel's gating row is core-local by construction.

Ingest: host exposes the bf16 voxel arrays as paired rows [S/2, 256] (a
reshape view) so every DMA run is 512B-contiguous; the device loads natural
-layout tiles, transposes them on the PE (bf16, 1 cycle/row) into bf16 PSUM
and stages them into a resident feature-major SBUF image. The within-tile
voxel order that falls out of the paired transpose is undone on the host.
Segment min/max run as bf16 TensorTensor chains (2x DVE mode). The fused
MLP runs in bf16; PSUM->SBUF relu-copies are balanced across DVE and Act.
Output is stored bf16 and upconverted on host (relu commutes with bf16
rounding). Weights load once; the rep loop (timing harness) re-runs stats +
MLP so consecutive reps pipeline across engines.
"""

import os
import sys

import numpy as np

for _p in ("/opt/trn_rl_repo",):
    if os.path.isdir(_p) and _p not in sys.path:
        sys.path.append(_p)

B = 8
L = 128
C = 128
OUT = 256
CA = 512
H = 170
VT = 512    # voxels per compute tile
CW = 2048   # voxels per stat chunk

STATS_GP = 0  # retained for test.py compatibility (unused)

_cache = {}

# within-tile column -> voxel permutation of the paired transpose:
# col (2a+j)*128 + p holds voxel 2*(a*128+p) + j
_PERM = np.empty(VT, np.int64)
for _a in range(2):
    for _j in range(2):
        for _pp in range(128):
            _PERM[(2 * _a + _j) * 128 + _pp] = 2 * (_a * 128 + _pp) + _j


def _build(S_pad, stats_gp=0, reps=1):
    from contextlib import ExitStack

    import concourse.bacc as bacc
    import concourse.mybir as mybir
    import concourse.tile as tile
    from concourse import masks

    f32 = mybir.dt.float32
    bf16 = mybir.dt.bfloat16
    Alu = mybir.AluOpType
    Act = mybir.ActivationFunctionType

    n_tiles = S_pad // VT
    cw = min(CW, S_pad)
    spans = []
    v0 = 0
    while v0 < S_pad:
        w = min(cw, S_pad - v0)
        spans.append((v0, w))
        v0 += w

    nc = bacc.Bacc("TRN2", target_bir_lowering=False, debug=False, num_devices=B)
    lidar = nc.dram_tensor("lidar", [S_pad // 2, 2 * L], bf16, kind="ExternalInput").ap()
    cam = nc.dram_tensor("cam", [S_pad // 2, 2 * C], bf16, kind="ExternalInput").ap()
    wl1 = nc.dram_tensor("W_l1", [CA, H], f32, kind="ExternalInput").ap()
    wl2 = nc.dram_tensor("W_l2", [H, L], f32, kind="ExternalInput").ap()
    wc1 = nc.dram_tensor("W_c1", [CA, H], f32, kind="ExternalInput").ap()
    wc2 = nc.dram_tensor("W_c2", [H, C], f32, kind="ExternalInput").ap()
    wf1 = nc.dram_tensor("W_f1", [2 * L, OUT], f32, kind="ExternalInput").ap()
    wf2 = nc.dram_tensor("W_f2", [OUT, OUT], f32, kind="ExternalInput").ap()
    out = nc.dram_tensor("out", [S_pad, OUT], bf16, kind="ExternalOutput").ap()

    with tile.TileContext(nc) as tc, ExitStack() as ctx:
        wpool = ctx.enter_context(tc.tile_pool(name="weights", bufs=1))
        respool = ctx.enter_context(tc.tile_pool(name="res", bufs=1))
        statpool = ctx.enter_context(tc.tile_pool(name="stat", bufs=1))
        natpool = ctx.enter_context(tc.tile_pool(name="nat", bufs=4))
        pstr = ctx.enter_context(tc.tile_pool(name="pstr", bufs=2, space="PSUM"))

        ident = wpool.tile([128, 128], bf16)
        identf = wpool.tile([128, 128], f32)
        masks.make_identity(nc, identf[:])
        nc.vector.tensor_copy(ident[:], identf[:])

        # --- weight SBUF tiles ---
        wf1_s = wpool.tile([128, 2, OUT], f32)
        wf2_stage = wpool.tile([128, 2, OUT], f32)
        wf2_s = wpool.tile([128, 2, OUT], bf16)
        w1e_s = wpool.tile([128, 2, OUT], bf16)
        wl1_s = wpool.tile([128, 4, H], f32)
        wc1_s = wpool.tile([128, 4, H], f32)
        wl2a_s = wpool.tile([128, L], f32)
        wl2b_s = wpool.tile([H - 128, L], f32)
        wc2a_s = wpool.tile([128, C], f32)
        wc2b_s = wpool.tile([H - 128, C], f32)

        nc.sync.dma_start(wl1_s[:], wl1.rearrange("(a p) h -> p a h", p=128))
        nc.sync.dma_start(wc1_s[:], wc1.rearrange("(a p) h -> p a h", p=128))
        nc.sync.dma_start(wl2a_s[:], wl2[0:128, :])
        nc.sync.dma_start(wl2b_s[:], wl2[128:H, :])
        nc.sync.dma_start(wc2a_s[:], wc2[0:128, :])
        nc.sync.dma_start(wc2b_s[:], wc2[128:H, :])
        nc.sync.dma_start(wf1_s[:], wf1.rearrange("(a p) o -> p a o", p=128))
        nc.sync.dma_start(wf2_stage[:], wf2.rearrange("(a p) o -> p a o", p=128))
        nc.scalar.activation(wf2_s[:], wf2_stage[:], Act.Copy)

        # preload activation tables off the critical path
        warm = wpool.tile([128, 1], f32)
        nc.vector.memset(warm[:], 0.0)
        nc.scalar.activation(warm[:], warm[:], Act.Relu)
        nc.scalar.activation(warm[:], warm[:], Act.Sigmoid)

        # resident feature-major voxel data
        xres = {
            "l": respool.tile([128, S_pad], bf16, name="xres_l", tag="xres_l"),
            "c": respool.tile([128, S_pad], bf16, name="xres_c", tag="xres_c"),
        }
        accbuf = {}
        for key in ("min_l", "max_l", "min_c", "max_c"):
            accbuf[key] = statpool.tile([128, cw], bf16, name="a" + key, tag="a" + key)

        def load_tile(t, which, src):
            """Paired-row load + PE transpose + staged copy into xres."""
            nat = natpool.tile([128, 2, 2 * L], bf16, name="nat", tag="nat" + which)
            nc.sync.dma_start(
                nat[:],
                src[t * 256 : (t + 1) * 256, :].rearrange("(a p) ff -> p a ff", p=128),
            )
            ps = pstr.tile([128, VT], bf16, name="pstr", tag="pstr")
            for a in range(2):
                for j in range(2):
                    nc.tensor.transpose(
                        ps[:, (2 * a + j) * 128 : (2 * a + j + 1) * 128],
                        nat[:, a, j * 128 : (j + 1) * 128],
                        ident[:],
                    )
            dst = xres[which][:, t * VT : (t + 1) * VT]
            # bf16->bf16 staged copy: cheap 2x path on DVE
            nc.vector.tensor_copy(dst, ps[:])

        for _rep in range(reps):
            rctx = ctx.enter_context(ExitStack())
            # ---- pass 1: loads + transposes + min/max stat chains ----
            for si, (v0, w) in enumerate(spans):
                for t in range(v0 // VT, (v0 + w) // VT):
                    load_tile(t, "l", lidar)
                    load_tile(t, "c", cam)
                for which in ("l", "c"):
                    dst = xres[which][:, v0 : v0 + w]
                    for statname, op in (("min", Alu.min), ("max", Alu.max)):
                        key = statname + "_" + which
                        acc = accbuf[key][:, 0:w]
                        if si == 0:
                            nc.gpsimd.tensor_copy(acc, dst)
                        else:
                            nc.vector.tensor_tensor(out=acc, in0=acc, in1=dst, op=op)

            # ---- finalize stats (per stream) interleaved with tiny-MLP L1
            # accumulation: the k-th matmul only needs the k-th stat ----
            with tc.tile_pool(name="pstiny", bufs=1, space="PSUM") as pstiny:
                tiny_ps = {}
                for name, w1_s in (("l", wl1_s), ("c", wc1_s)):
                    for tag, mo, mn in (("h1a", 0, 128), ("h1b", 128, H - 128)):
                        tiny_ps[name, tag] = pstiny.tile(
                            [mn, 1], f32, name="tiny" + tag + name, tag=tag + name
                        )
                for k, key in enumerate(("min_l", "max_l", "min_c", "max_c")):
                    op = Alu.min if key.startswith("min") else Alu.max
                    w = cw
                    while w > 128:
                        h = w // 2
                        nc.vector.tensor_tensor(
                            out=accbuf[key][:, 0:h],
                            in0=accbuf[key][:, 0:h],
                            in1=accbuf[key][:, h:w],
                            op=op,
                        )
                        w = h
                    s = statpool.tile([128, 1], f32, tag="stat" + key)
                    nc.vector.tensor_reduce(
                        s[:], accbuf[key][:, 0:w], axis=mybir.AxisListType.X, op=op
                    )
                    if k == 0:
                        # warmup train: keeps PE continuously busy through the
                        # stat finalization so pass 2 starts at full clock
                        wps = pstiny.tile([128, VT], f32, name="wps", tag="warm_ps")
                        for _ in range(8):
                            nc.tensor.matmul(
                                wps[:], accbuf[key][:, 0:128],
                                accbuf[key][:, 0:VT], start=True, stop=True,
                            )
                    for name, w1_s in (("l", wl1_s), ("c", wc1_s)):
                        for tag, mo, mn in (("h1a", 0, 128), ("h1b", 128, H - 128)):
                            nc.tensor.matmul(
                                tiny_ps[name, tag][:],
                                w1_s[:, k, mo : mo + mn],
                                s[:],
                                start=(k == 0),
                                stop=(k == 3),
                            )

                def tiny_head(w2a_s, w2b_s, name):
                    h1_sb = []
                    for tag, mo, mn in (("h1a", 0, 128), ("h1b", 128, H - 128)):
                        hs = statpool.tile([mn, 1], f32, tag=tag + "s" + name)
                        nc.scalar.activation(hs[:], tiny_ps[name, tag][:], Act.Relu)
                        h1_sb.append(hs)
                    att_ps = pstiny.tile([128, 1], f32, name="attps", tag="attps" + name)
                    nc.tensor.matmul(
                        att_ps[:], w2a_s[:], h1_sb[0][:], start=True, stop=False
                    )
                    nc.tensor.matmul(
                        att_ps[:], w2b_s[:], h1_sb[1][:], start=False, stop=True
                    )
                    att_r = statpool.tile([128, 1], f32, tag="attr" + name)
                    nc.scalar.activation(att_r[:], att_ps[:], Act.Relu)
                    att = statpool.tile([128, 1], f32, tag="att" + name)
                    nc.scalar.activation(att[:], att_r[:], Act.Sigmoid)
                    return att

                att_l = tiny_head(wl2a_s, wl2b_s, "l")
                att_c = tiny_head(wc2a_s, wc2b_s, "c")

            nc.scalar.activation(
                w1e_s[:, 0, :], wf1_s[:, 0, :], Act.Copy, scale=att_l[:]
            )
            nc.scalar.activation(
                w1e_s[:, 1, :], wf1_s[:, 1, :], Act.Copy, scale=att_c[:]
            )

            # ---- pass 2: big gated MLP (software-pipelined PE stream) ----
            psl1 = rctx.enter_context(tc.tile_pool(name="psl1", bufs=2, space="PSUM"))
            psl2 = rctx.enter_context(tc.tile_pool(name="psl2", bufs=2, space="PSUM"))
            h1pool = rctx.enter_context(tc.tile_pool(name="h1", bufs=2))
            outpool = rctx.enter_context(tc.tile_pool(name="outp", bufs=2))

            def emit_l1(t):
                """L1 matmuls for tile t -> 2-bank PSUM tile + one wide copy."""
                xt_l = xres["l"][:, t * VT : (t + 1) * VT]
                xt_c = xres["c"][:, t * VT : (t + 1) * VT]
                h1t = h1pool.tile([128, 2 * VT], bf16, tag="h1")
                ps1 = psl1.tile([128, 2 * VT], f32, name="ps1", tag="psl1")
                for m in range(2):
                    sl = ps1[:, m * VT : (m + 1) * VT]
                    nc.tensor.matmul(
                        sl, w1e_s[:, 0, m * 128 : (m + 1) * 128], xt_l,
                        start=True, stop=False,
                    )
                    nc.tensor.matmul(
                        sl, w1e_s[:, 1, m * 128 : (m + 1) * 128], xt_c,
                        start=False, stop=True,
                    )
                nc.scalar.activation(h1t[:], ps1[:], Act.Relu)
                return h1t

            def emit_l2(t, h1t, ob2, slot):
                for g in range(2):
                    ps2 = psl2.tile([128, 2, OUT], f32, name="ps2", tag="psl2")
                    for h in range(2):
                        v = g * 2 + h
                        sl = ps2[:, h, :]
                        nc.tensor.matmul(
                            sl, h1t[:, v * 128 : (v + 1) * 128], wf2_s[:, 0, :],
                            start=True, stop=False,
                        )
                        nc.tensor.matmul(
                            sl, h1t[:, VT + v * 128 : VT + (v + 1) * 128],
                            wf2_s[:, 1, :], start=False, stop=True,
                        )
                    obs = ob2[:, slot, g * 2 : (g + 1) * 2, :]
                    if g == 1 and t % 5 == 0:
                        nc.vector.tensor_scalar_max(obs, ps2[:], 0.0)
                    else:
                        nc.scalar.activation(obs, ps2[:], Act.Relu)

            GRP = 4
            ob2 = None
            h1cur = emit_l1(0)
            for t in range(n_tiles):
                h1next = emit_l1(t + 1) if t + 1 < n_tiles else None
                slot = t % GRP
                nslot = min(GRP, n_tiles - (t - slot))
                if slot == 0:
                    ob2 = outpool.tile([128, GRP, 4, OUT], bf16, tag="ob2")
                emit_l2(t, h1cur, ob2, slot)
                h1cur = h1next
                if slot == nslot - 1:
                    r0 = (t - slot) * VT
                    nc.sync.dma_start(
                        out[r0 : r0 + nslot * VT, :].rearrange(
                            "(a p) f -> p a f", p=128
                        ),
                        ob2[:, 0:nslot, :, :].rearrange("p b a f -> p (b a) f"),
                    )

            rctx.close()

    nc.compile()
    return nc


def _get_program(S_pad):
    if S_pad not in _cache:
        _cache[S_pad] = _build(S_pad)
    return _cache[S_pad]


def _to_bf16(a):
    import ml_dtypes

    return np.asarray(a, np.float32).astype(ml_dtypes.bfloat16)


def shard_inputs(lidar, cam, batch_idx, W_l1, W_l2, W_c1, W_c2, W_f1, W_f2):
    """Split by batch (batch_idx sorted), pad with replicated real rows."""
    lidar = _to_bf16(lidar)
    cam = _to_bf16(cam)
    batch_idx = np.asarray(batch_idx)
    bounds = np.searchsorted(batch_idx, np.arange(B + 1))
    sizes = np.diff(bounds)
    S_pad = int(-(-max(int(sizes.max()), 1) // VT) * VT)
    weights = {
        "W_l1": np.ascontiguousarray(W_l1, np.float32),
        "W_l2": np.ascontiguousarray(W_l2, np.float32),
        "W_c1": np.ascontiguousarray(W_c1, np.float32),
        "W_c2": np.ascontiguousarray(W_c2, np.float32),
        "W_f1": np.ascontiguousarray(W_f1, np.float32),
        "W_f2": np.ascontiguousarray(W_f2, np.float32),
    }
    in_maps = []
    for b in range(B):
        s0, s1 = int(bounds[b]), int(bounds[b + 1])
        n = s1 - s0
        l = np.empty((S_pad, L), lidar.dtype)
        c = np.empty((S_pad, C), cam.dtype)
        if n > 0:
            l[:n] = lidar[s0:s1]
            c[:n] = cam[s0:s1]
            l[n:] = lidar[s1 - 1]
            c[n:] = cam[s1 - 1]
        else:
            l[:] = 0
            c[:] = 0
        in_maps.append(
            {
                "lidar": l.reshape(S_pad // 2, 2 * L),
                "cam": c.reshape(S_pad // 2, 2 * C),
                **weights,
            }
        )
    return in_maps, bounds, sizes, S_pad


def unpermute(res_out, S_pad):
    """Undo the within-tile voxel permutation of the paired transpose."""
    r = np.asarray(res_out).reshape(S_pad // VT, VT, OUT)
    outp = np.empty_like(r)
    outp[:, _PERM, :] = r
    return outp.reshape(S_pad, OUT)


def kernel(lidar, cam, batch_idx, W_l1, W_l2, W_c1, W_c2, W_f1, W_f2):
    from concourse.bass_utils import run_bass_kernel_spmd

    in_maps, bounds, sizes, S_pad = shard_inputs(
        lidar, cam, batch_idx, W_l1, W_l2, W_c1, W_c2, W_f1, W_f2
    )
    nc = _get_program(S_pad)
    res = run_bass_kernel_spmd(nc, in_maps, core_ids=list(range(B)))
    N = lidar.shape[0]
    out_full = np.empty((N, OUT), np.float32)
    for b in range(B):
        s0, s1 = int(bounds[b]), int(bounds[b + 1])
        if s1 > s0:
            ob = unpermute(res.results[b]["out"], S_pad).astype(np.float32)
            out_full[s0:s1] = ob[: s1 - s0]
    return out_full
